# revision 1
# baseline (speedup 1.0000x reference)
"""Swin-style window attention (nn_BasicAttentionBlock) on 8 trn2 NeuronCores.

Strategy (data-parallel over the 4096 windows, 512/core):
- Host pre-packs per-core operands in bf16:
    qT/kT : channel-major per (window, head) -> per-head 32x49 matmul operands
    vx    : [49, W*16*33] v blocks with a ones-column appended (row-sums fall
            out of the AV matmul for free)
    eb    : exp(bias^T + mask^T) combined multiplicative softmax table
- Device per window w, head h (16 heads, d=32):
    pT[m,n] = sum_d k[m,d] q[n,d]        (matmul, K=32, out 49x49 in PSUM)
    p = exp(scale * pT) * eb[w,h]        (ACT exp reading 2 PSUM banks/window,
                                          DVE bf16 multiply)
    o[n,c]  = sum_m p[m,n] * vx[m,c]     (matmul, K=49; col 32 = row sum s[n])
    out     = o[:, :32] * (1/s)          (DVE reciprocal + tensor mul,
                                          writes fp32, DMA'd out)
- Softmax skips the max-subtraction: scores*scale ~ N(0,1), exp is safe in
  fp32, and exp(-1e9 mask) underflows to exactly 0.
"""

import os
from contextlib import ExitStack

import numpy as np
import ml_dtypes

WINDOW = 7
N = 49
C = 512
H = 16
D = 32
NW = 1024
B_ = 4096
NCORES = 8
W = B_ // NCORES          # 512 windows per core
SCALE = float(D) ** -0.5

WCH = 16                  # windows per input DMA chunk
OCH = 8                   # windows per output DMA chunk

BF16 = ml_dtypes.bfloat16
# praw column-slot -> head: QK emitted j-major with bank=j%2, slot=4*(j//2)+g
HPERM = [4 * g + j for j in range(4) for g in range(4)]


def _build_nc(Wn: int):
    import concourse.bass as bass
    import concourse.tile as tile
    import concourse.bacc as bacc
    from concourse import mybir

    bf = mybir.dt.bfloat16
    f32 = mybir.dt.float32

    nc = bacc.Bacc(None, target_bir_lowering=False)
    qk = nc.dram_tensor("qk", [128, Wn * 392], bf, kind="ExternalInput")
    ve = nc.dram_tensor("ve", [49, Wn * 1312], bf, kind="ExternalInput")
    outp = nc.dram_tensor("outp", [Wn * 49, 512], f32, kind="ExternalOutput")

    nwch = (Wn + WCH - 1) // WCH
    Exp = mybir.ActivationFunctionType.Exp

    with tile.TileContext(nc) as tc, ExitStack() as ctx:
        in_pool = ctx.enter_context(tc.tile_pool(name="inp", bufs=2))
        p_pool = ctx.enter_context(tc.tile_pool(name="p", bufs=3))
        r_pool = ctx.enter_context(tc.tile_pool(name="r", bufs=3))
        o_pool = ctx.enter_context(tc.tile_pool(name="o", bufs=2))
        ps_qk = ctx.enter_context(tc.tile_pool(name="psqk", bufs=1, space="PSUM"))
        ps_av = ctx.enter_context(tc.tile_pool(name="psav", bufs=2, space="PSUM"))

        for ci in range(nwch):
            wlo = ci * WCH
            nwin = min(WCH, Wn - wlo)
            qk_sb = in_pool.tile([128, nwin * 392], bf, tag="qk")
            nc.sync.dma_start(qk_sb[:], qk[:, wlo * 392:(wlo + nwin) * 392])
            ve_sb = in_pool.tile([49, nwin * 1312], bf, tag="ve")
            nc.sync.dma_start(ve_sb[:], ve[:, wlo * 1312:(wlo + nwin) * 1312])

            for wi in range(nwin):
                w = wlo + wi
                # ---- QK^T: 16 per-head matmuls into one 2-bank PSUM tile
                pq = ps_qk.tile([49, 2048], f32, tag="pq")
                for j in range(4):
                    for g in range(4):
                        colq = wi * 392 + g * 49
                        colk = wi * 392 + 196 + g * 49
                        qcol = 512 * j + 49 * g
                        nc.tensor.matmul(
                            pq[:, qcol:qcol + 49],
                            qk_sb[32 * j:32 * j + 32, colk:colk + 49],
                            qk_sb[32 * j:32 * j + 32, colq:colq + 49],
                            start=True, stop=True,
                            tile_position=(32 * j, 0),
                        )
                # ---- softmax numerator: exp over both banks in one ACT op,
                # then multiply by exp(bias+mask) table (bf16, DVE 2x mode)
                praw = p_pool.tile([49, 784], bf, tag="praw")
                pq_b = pq[:]
                pq_ap = bass.AP(pq_b.tensor, pq_b.offset,
                                [pq_b.ap[0], [512, 4], [1, 196]])
                nc.scalar.activation(praw[:].rearrange("p (b f) -> p b f", b=4),
                                     pq_ap, Exp, scale=SCALE)
                pmul = p_pool.tile([49, 784], bf, tag="pmul")
                ebo = wi * 1312 + 528
                nc.vector.tensor_mul(pmul[:], praw[:],
                                     ve_sb[:, ebo:ebo + 784])
                # ---- AV: 16 per-head matmuls; vx has a ones column so
                # out[:, 33h+32] = row sum
                av = ps_av.tile([49, 1024], f32, tag="av")
                for s in range(16):
                    h = HPERM[s]
                    acol = 33 * h if h < 8 else 512 + 33 * (h - 8)
                    nc.tensor.matmul(
                        av[:, acol:acol + 33],
                        pmul[:, 49 * s:49 * s + 49],
                        ve_sb[:, wi * 1312 + h * 33:wi * 1312 + h * 33 + 33],
                        start=True, stop=True,
                    )
                # ---- normalize: recip of sums, multiply, write fp32
                av_ap = av[:]
                rec = r_pool.tile([49, 16], f32, tag="rec")
                sums0 = bass.AP(av_ap.tensor, av_ap.offset + 32, [av_ap.ap[0], [33, 8]])
                sums1 = bass.AP(av_ap.tensor, av_ap.offset + 512 + 32, [av_ap.ap[0], [33, 8]])
                nc.vector.reciprocal(rec[:, 0:8], sums0)
                nc.vector.reciprocal(rec[:, 8:16], sums1)

                if w % OCH == 0:
                    o_sb = o_pool.tile([49, OCH * 512], f32, tag="osb")
                o_off = (w % OCH) * 512
                o_ap = o_sb[:]
                rec_ap = rec[:]
                for b in range(2):
                    out_dst = bass.AP(o_ap.tensor, o_ap.offset + o_off + 256 * b,
                                      [o_ap.ap[0], [32, 8], [1, 32]])
                    vals = bass.AP(av_ap.tensor, av_ap.offset + 512 * b,
                                   [av_ap.ap[0], [33, 8], [1, 32]])
                    rbc = bass.AP(rec_ap.tensor, rec_ap.offset + 8 * b,
                                  [rec_ap.ap[0], [1, 8], [0, 32]])
                    nc.vector.tensor_mul(out_dst, vals, rbc)

                if w % OCH == OCH - 1 or w == Wn - 1:
                    nlast = (w % OCH) + 1
                    base = (w - nlast + 1) * 49 * 512
                    dst = bass.AP(outp[:].tensor, base,
                                  [[512, 49], [49 * 512, nlast], [1, 512]])
                    src = bass.AP(o_ap.tensor, o_ap.offset,
                                  [o_ap.ap[0], [512, nlast], [1, 512]])
                    nc.sync.dma_start(dst, src)
    nc.compile()
    return nc


def _pack_inputs(q, k, v, bias_table, mask, rel_index):
    """Host-side packing into per-core bf16 operand tables."""
    Wn = W
    # bias^T and mask^T tables (pT layout: rows=m key token, cols=n query)
    bias = np.asarray(bias_table)[np.asarray(rel_index)]        # [n, m, H]
    biasT = bias.transpose(2, 1, 0).astype(np.float32)          # [H, m, n]
    maskT = np.asarray(mask).transpose(0, 2, 1).astype(np.float32)  # [NW, m, n]
    # combined multiplicative table for the 1024 unique windows
    ebu = np.exp(biasT[None] + maskT[:, None]).astype(BF16)     # [NW, H, m, n]

    qr = np.asarray(q).reshape(B_, N, 4, 4, 32)
    kr = np.asarray(k).reshape(B_, N, 4, 4, 32)
    vr = np.asarray(v).reshape(B_, N, H, D)

    in_maps = []
    for c in range(NCORES):
        sl = slice(c * Wn, (c + 1) * Wn)
        # [w,n,g,j,d] -> [j,d,w,g,n]; qk col layout per w: [q 196 | k 196]
        qkh = np.empty((128, Wn, 392), dtype=BF16)
        qkh[:, :, :196] = qr[sl].transpose(3, 4, 0, 2, 1).reshape(128, Wn, 196)
        qkh[:, :, 196:] = kr[sl].transpose(3, 4, 0, 2, 1).reshape(128, Wn, 196)
        vx = np.ones((N, Wn, H, 33), dtype=BF16)
        vx[:, :, :, :32] = vr[sl].transpose(1, 0, 2, 3).astype(BF16)
        widx = (np.arange(c * Wn, (c + 1) * Wn)) % NW
        ebc = ebu[widx][:, HPERM].transpose(2, 0, 1, 3)          # [m, W, slot, n]
        veh = np.empty((N, Wn, 1312), dtype=BF16)
        veh[:, :, :528] = vx.reshape(N, Wn, 528)
        veh[:, :, 528:] = ebc.reshape(N, Wn, 784)
        in_maps.append({
            "qk": np.ascontiguousarray(qkh.reshape(128, Wn * 392)),
            "ve": np.ascontiguousarray(veh.reshape(N, Wn * 1312)),
        })
    return in_maps


_CACHE = {}


def kernel(q, k, v, bias_table, mask, rel_index):
    from concourse.bass_utils import run_bass_kernel_spmd

    in_maps = _pack_inputs(q, k, v, bias_table, mask, rel_index)
    if "nc" not in _CACHE:
        _CACHE["nc"] = _build_nc(W)
    nc = _CACHE["nc"]
    trace = bool(int(os.environ.get("KBENCH_TRACE", "0")))
    res = run_bass_kernel_spmd(nc, in_maps, core_ids=list(range(NCORES)),
                               trace=trace)
    if trace:
        _CACHE["exec_time_ns"] = res.exec_time_ns
        _CACHE["results"] = res
    out = np.empty((B_, N, C), dtype=np.float32)
    for c in range(NCORES):
        out[c * W:(c + 1) * W] = res.results[c]["outp"].reshape(W, N, C)
    return out

